# revision 6
# baseline (speedup 1.0000x reference)
"""LIF (leaky integrate-and-fire) scan kernel for Trainium2, 8 NeuronCores.

Reference semantics (fp32, T=8 innermost axis):
    mem = 0
    for t in range(T):
        mem = mem * 0.5 + x[..., t]
        s[..., t] = (mem >= 1.0)
        mem = mem * (1.0 - s[..., t])

Sharding: data-parallel over the leading dim (64 -> 8 per core). Per core the
input is viewed as [128 partitions, 8192 neurons, 8 timesteps]; the scan is
elementwise over neurons and sequential over t.

Per chunk of neurons, on the Vector engine (all ops exact in fp32):
    m   = (m  mult 0.5) add x_t        # scalar_tensor_tensor
    x_t = (m  is_ge 1.0)               # tensor_scalar, spike written in place
    m   = (m  is_lt 1.0) mult m        # scalar_tensor_tensor (reset)
The spike overwrites the consumed x_t slot, so the x tile doubles as the
output tile and is DMA'd back out when all 8 steps are done.
"""

import numpy as np

import concourse.bass as bass
import concourse.tile as tile
from concourse import bacc, mybir
from concourse.bass_utils import run_bass_kernel_spmd

P = 128          # SBUF partitions
T = 8            # timesteps (innermost axis)
NPB = 8192       # neurons per partition per core: 8*128*32*32 / 128
FREE = NPB * T   # fp32 elements per partition per core
CH = 1024        # neurons per chunk (per partition)
NCH = NPB // CH

THRESH = 1.0
DECAY = 0.5
F32 = mybir.dt.float32
N_CORES = 8

Alu = mybir.AluOpType


def _build() -> bass.Bass:
    nc = bacc.Bacc("TRN2", target_bir_lowering=False, debug=False)
    x = nc.dram_tensor("x", [P, FREE], F32, kind="ExternalInput").ap()
    y = nc.dram_tensor("y", [P, FREE], F32, kind="ExternalOutput").ap()

    with tile.TileContext(nc) as tc:
        with (
            tc.tile_pool(name="io", bufs=2) as io,
            tc.tile_pool(name="state", bufs=2) as state,
        ):
            for c in range(NCH):
                xt = io.tile([P, CH * T], F32, tag="xt")
                nc.gpsimd.dma_start(xt[:], x[:, bass.ts(c, CH * T)])
                x3 = xt.rearrange("p (n t) -> p n t", t=T)
                ot = io.tile([P, CH * T], F32, tag="ot")
                o3 = ot.rearrange("p (n t) -> p n t", t=T)
                m = state.tile([P, CH], F32, tag="m")
                for t in range(T):
                    xs = x3[:, :, t]
                    os = o3[:, :, t]
                    if t == 0:
                        # mem0 = 0, so m = x_0 after decay+add.
                        if T > 1:
                            nc.vector.scalar_tensor_tensor(
                                m[:], xs, THRESH, xs, Alu.is_lt, Alu.mult
                            )
                        nc.vector.tensor_scalar(
                            os, xs, THRESH, None, Alu.is_ge, Alu.bypass
                        )
                        continue
                    nc.vector.scalar_tensor_tensor(
                        m[:], m[:], DECAY, xs, Alu.mult, Alu.add
                    )
                    nc.vector.tensor_scalar(
                        os, m[:], THRESH, None, Alu.is_ge, Alu.bypass
                    )
                    if t < T - 1:
                        nc.vector.scalar_tensor_tensor(
                            m[:], m[:], THRESH, m[:], Alu.is_lt, Alu.mult
                        )
                nc.gpsimd.dma_start(y[:, bass.ts(c, CH * T)], ot[:])
    nc.compile()
    return nc


_NC_CACHE: bass.Bass | None = None


def _get_nc() -> bass.Bass:
    global _NC_CACHE
    if _NC_CACHE is None:
        _NC_CACHE = _build()
    return _NC_CACHE


def _run(X: np.ndarray, **spmd_kwargs):
    assert X.shape == (64, 128, 32, 32, 8), X.shape
    X = np.ascontiguousarray(X, dtype=np.float32)
    per_core = 64 // N_CORES
    in_maps = [
        {"x": X[i * per_core : (i + 1) * per_core].reshape(P, FREE)}
        for i in range(N_CORES)
    ]
    res = run_bass_kernel_spmd(
        _get_nc(), in_maps, core_ids=list(range(N_CORES)), **spmd_kwargs
    )
    out = np.empty_like(X)
    for i, r in enumerate(res.results):
        out[i * per_core : (i + 1) * per_core] = r["y"].reshape(
            per_core, 128, 32, 32, 8
        )
    return out, res


def kernel(X: np.ndarray) -> np.ndarray:
    out, _ = _run(X)
    return out


# revision 8
# speedup vs baseline: 1.3603x; 1.3603x over previous
"""LIF (leaky integrate-and-fire) scan kernel for Trainium2, 8 NeuronCores.

Reference semantics (fp32, T=8 innermost axis):
    mem = 0
    for t in range(T):
        mem = mem * 0.5 + x[..., t]
        s[..., t] = (mem >= 1.0)
        mem = mem * (1.0 - s[..., t])

Sharding: data-parallel over the leading dim (64 -> 8 per core). On the host,
each core's shard is transposed to a t-major layout [128 partitions, T=8,
8192 neurons] so that every per-timestep slice the device touches is
contiguous (strided SBUF reads measured ~2x slower on DVE, and strided writes
block the 2x tensor_scalar mode).

Per chunk of neurons, all on the Vector engine (exact in fp32):
    m    = (m  mult 0.5) add x_t       # scalar_tensor_tensor, 1x
    x_t  = (m  is_ge 1.0)              # tensor_scalar spike, 2x, in place
    m    = (m  is_lt 1.0) mult m       # scalar_tensor_tensor reset, 1x
Each timestep's strip is loaded/stored with its own ~1 MiB DMA so loads,
compute, and stores pipeline at strip granularity.
"""

import numpy as np

import concourse.bass as bass
import concourse.tile as tile
from concourse import bacc, mybir
from concourse.bass_utils import run_bass_kernel_spmd

P = 128          # SBUF partitions
T = 8            # timesteps (innermost axis of the original input)
NPB = 8192       # neurons per partition per core: 8*128*32*32 / 128
FREE = NPB * T   # fp32 elements per partition per core
CH = 2048        # neurons per chunk (per partition)
NCH = NPB // CH

THRESH = 1.0
DECAY = 0.5
F32 = mybir.dt.float32
N_CORES = 8

Alu = mybir.AluOpType


def _build() -> bass.Bass:
    nc = bacc.Bacc("TRN2", target_bir_lowering=False, debug=False)
    # t-major per core: x[p, t*NPB + n]
    x = nc.dram_tensor("x", [P, FREE], F32, kind="ExternalInput").ap()
    y = nc.dram_tensor("y", [P, FREE], F32, kind="ExternalOutput").ap()

    with tile.TileContext(nc) as tc:
        with (
            tc.tile_pool(name="strips", bufs=8) as strips,
            tc.tile_pool(name="state", bufs=2) as state,
        ):
            for c in range(NCH):
                xs = []
                for t in range(T):
                    st = strips.tile([P, CH], F32, tag="strip", name=f"st{c}_{t}")
                    nc.gpsimd.dma_start(
                        st[:], x[:, t * NPB + c * CH : t * NPB + (c + 1) * CH]
                    )
                    xs.append(st)
                m = state.tile([P, CH], F32, tag="m", name=f"m{c}")
                for t in range(T):
                    st = xs[t]
                    if t == 0:
                        # mem0 = 0, so m = x_0 after decay+add.
                        nc.vector.scalar_tensor_tensor(
                            m[:], st[:], THRESH, st[:], Alu.is_lt, Alu.mult
                        )
                        nc.vector.tensor_scalar(
                            st[:], st[:], THRESH, None, Alu.is_ge, Alu.bypass
                        )
                    else:
                        nc.vector.scalar_tensor_tensor(
                            m[:], m[:], DECAY, st[:], Alu.mult, Alu.add
                        )
                        nc.vector.tensor_scalar(
                            st[:], m[:], THRESH, None, Alu.is_ge, Alu.bypass
                        )
                        if t < T - 1:
                            nc.vector.scalar_tensor_tensor(
                                m[:], m[:], THRESH, m[:], Alu.is_lt, Alu.mult
                            )
                    nc.gpsimd.dma_start(
                        y[:, t * NPB + c * CH : t * NPB + (c + 1) * CH], st[:]
                    )
    nc.compile()
    return nc


_NC_CACHE: bass.Bass | None = None


def _get_nc() -> bass.Bass:
    global _NC_CACHE
    if _NC_CACHE is None:
        _NC_CACHE = _build()
    return _NC_CACHE


def _run(X: np.ndarray, **spmd_kwargs):
    assert X.shape == (64, 128, 32, 32, 8), X.shape
    X = np.ascontiguousarray(X, dtype=np.float32)
    per_core = 64 // N_CORES
    # [core, p, n, t] -> t-major [core, p, t, n], contiguous per core
    Xt = np.ascontiguousarray(
        X.reshape(N_CORES, P, NPB, T).transpose(0, 1, 3, 2)
    )
    in_maps = [{"x": Xt[i].reshape(P, FREE)} for i in range(N_CORES)]
    res = run_bass_kernel_spmd(
        _get_nc(), in_maps, core_ids=list(range(N_CORES)), **spmd_kwargs
    )
    out = np.empty_like(X)
    for i, r in enumerate(res.results):
        # t-major [p, t, n] -> [p, n, t] -> original shard shape
        s = r["y"].reshape(P, T, NPB).transpose(0, 2, 1)
        out[i * per_core : (i + 1) * per_core] = s.reshape(
            per_core, 128, 32, 32, 8
        )
    return out, res


def kernel(X: np.ndarray) -> np.ndarray:
    out, _ = _run(X)
    return out
